# revision 3
# baseline (speedup 1.0000x reference)
"""DiffVolume Trainium2 kernel.

volume[b, c, d, h, w] = left[b, c, h, w] - right[b, c, h, w - d]  (0 where w < d)

Shapes (hardcoded): left/right (2, 32, 96, 320) f32, D = 48.
Sharding: flatten (b, c) -> bc = 64, shard bc across 8 cores (8 bc each).
Each core reads its (8, 96, 320) input shards and writes its (8, 48, 96, 320)
output chunk; chunks concatenate on bc to the full volume.

Per-core kernel layout:
 - 768 rows (bc, h) -> 6 blocks of 128 partitions (row r = t*128 + p).
 - left/right resident in SBUF as [128, 6*320].
 - Output staged in SBUF tiles [128, G*6*320] (G=8 disparities per group,
   double-buffered): one tensor_sub per disparity covering all 6 blocks via a
   2D free-dim AP (shifted read of right), plus a memset for the w < d zeros.
 - HWDGE DMA out per (group, block, bc-piece) back to DRAM.
"""

import numpy as np

MAX_DISP = 48
B, C, H, W = 2, 32, 96, 320
NCORES = 8
BC = B * C                 # 64
BC_PER = BC // NCORES      # 8 bc rows per core
ROWS = BC_PER * H          # 768
P = 128
NT = ROWS // P             # 6 row blocks
G = 8                      # disparities per output tile
NG = MAX_DISP // G         # 6 groups

_NC_CACHE = {}


def _pieces(t):
    """Split block t's 128 partitions into runs with constant bc.

    Returns list of (p0, p1, bc, h0): rows r = t*128 + p, bc = r // H, h = r % H.
    """
    res = []
    r0 = t * P
    r = r0
    while r < r0 + P:
        bc = r // H
        r_end = min((bc + 1) * H, r0 + P)
        res.append((r - r0, r_end - r0, bc, r % H))
        r = r_end
    return res


def _build_nc():
    import concourse.bass as bass
    import concourse.bacc as bacc
    import concourse.mybir as mybir
    from concourse import tile

    f32 = mybir.dt.float32
    nc = bacc.Bacc("TRN2")
    left = nc.dram_tensor("left", [BC_PER, H, W], f32, kind="ExternalInput")
    right = nc.dram_tensor("right", [BC_PER, H, W], f32, kind="ExternalInput")
    out = nc.dram_tensor("out", [BC_PER, MAX_DISP, H, W], f32, kind="ExternalOutput")

    with tile.TileContext(nc) as tc:
        with tc.tile_pool(name="io", bufs=1) as iop, tc.tile_pool(
            name="op", bufs=2
        ) as outp:
            lt = iop.tile([P, NT * W], f32)
            rt = iop.tile([P, NT * W], f32)
            l3 = lt[:].rearrange("p (t w) -> p t w", t=NT, w=W)
            r3 = rt[:].rearrange("p (t w) -> p t w", t=NT, w=W)
            lsrc = left[:].rearrange("bc h w -> (bc h) w").rearrange(
                "(t p) w -> p t w", p=P
            )
            rsrc = right[:].rearrange("bc h w -> (bc h) w").rearrange(
                "(t p) w -> p t w", p=P
            )
            nc.sync.dma_start(out=l3, in_=lsrc)
            nc.sync.dma_start(out=r3, in_=rsrc)

            for dg in range(NG):
                d0 = dg * G
                ot = outp.tile([P, G * NT * W], f32, tag="out")
                o4 = ot[:].rearrange("p (g t w) -> p g t w", g=G, t=NT, w=W)
                for g in range(G):
                    d = d0 + g
                    if d:
                        nc.gpsimd.memset(o4[:, g, :, 0:d], 0.0)
                    nc.vector.tensor_sub(
                        o4[:, g, :, d:W], l3[:, :, d:W], r3[:, :, 0 : W - d]
                    )
                for t in range(NT):
                    for p0, p1, bc, h0 in _pieces(t):
                        dest = out[bc, d0 : d0 + G, h0 : h0 + (p1 - p0), :].rearrange(
                            "d h w -> h d w"
                        )
                        nc.sync.dma_start(out=dest, in_=o4[p0:p1, :, t, :])
    nc.finalize()
    return nc


def _get_nc():
    if "nc" not in _NC_CACHE:
        _NC_CACHE["nc"] = _build_nc()
    return _NC_CACHE["nc"]


def run(left_feature, right_feature, **spmd_kwargs):
    """Run the SPMD kernel; returns (volume, BassKernelResults)."""
    from concourse.bass_utils import run_bass_kernel_spmd

    nc = _get_nc()
    lf = np.ascontiguousarray(np.asarray(left_feature), dtype=np.float32).reshape(
        BC, H, W
    )
    rf = np.ascontiguousarray(np.asarray(right_feature), dtype=np.float32).reshape(
        BC, H, W
    )
    in_maps = [
        {
            "left": np.ascontiguousarray(lf[k * BC_PER : (k + 1) * BC_PER]),
            "right": np.ascontiguousarray(rf[k * BC_PER : (k + 1) * BC_PER]),
        }
        for k in range(NCORES)
    ]
    res = run_bass_kernel_spmd(nc, in_maps, core_ids=list(range(NCORES)), **spmd_kwargs)
    chunks = [res.results[k]["out"] for k in range(NCORES)]
    vol = np.concatenate(chunks, axis=0).reshape(B, C, MAX_DISP, H, W)
    return vol, res


def kernel(left_feature, right_feature):
    vol, _ = run(left_feature, right_feature)
    return vol


# revision 8
# speedup vs baseline: 1.1708x; 1.1708x over previous
"""DiffVolume Trainium2 kernel.

volume[b, c, d, h, w] = left[b, c, h, w] - right[b, c, h, w - d]  (0 where w < d)

Shapes (hardcoded): left/right (2, 32, 96, 320) f32, D = 48.
Sharding: flatten (b, c) -> bc = 64, shard bc across 8 cores (8 bc each).
Each core reads its (8, 96, 320) input shards and writes its (8, 48, 96, 320)
output chunk; chunks concatenate on bc to the full volume.

Per-core kernel layout:
 - 768 rows (bc, h) -> 6 blocks of 128 partitions (row r = t*128 + p).
 - left/right resident in SBUF as [128, 6*320], loaded block-by-block so
   compute starts after the first block lands.
 - Disparities processed in groups (small leading groups shorten the startup
   ramp). Group tile [128, G*6*320], double-buffered. One tensor_sub per
   disparity covers all 6 blocks via a 2D free-dim AP (shifted read of right).
 - Only w >= d0 is written back (d0 = group's first disparity): the PJRT/NEFF
   output buffers are zero-initialized and donated, so the w < d0 region of
   the output stays 0 without being written. Inside a group, the small
   parallelogram d0 <= w < d is zeroed in SBUF via memset before the DMA.
 - HWDGE DMA out per (group, block, bc-piece) back to DRAM.
"""

import numpy as np

MAX_DISP = 48
B, C, H, W = 2, 32, 96, 320
NCORES = 8
BC = B * C                 # 64
BC_PER = BC // NCORES      # 8 bc rows per core
ROWS = BC_PER * H          # 768
P = 128
NT = ROWS // P             # 6 row blocks
GROUPS = (4,) * 12             # disparity group sizes, sum = 48
GMAX = max(GROUPS)
OUT_BUFS = 3
SPLIT_FIRST = True

_NC_CACHE = {}


def _pieces(t):
    """Split block t's 128 partitions into runs with constant bc.

    Returns list of (p0, p1, bc, h0): rows r = t*128 + p, bc = r // H, h = r % H.
    """
    res = []
    r0 = t * P
    r = r0
    while r < r0 + P:
        bc = r // H
        r_end = min((bc + 1) * H, r0 + P)
        res.append((r - r0, r_end - r0, bc, r % H))
        r = r_end
    return res


def build_body(nc, tc, left, right, out, rep=1):
    """Emit the kernel body. rep>1 re-runs the group loop (for benchmarks)."""
    import concourse.mybir as mybir

    f32 = mybir.dt.float32
    with tc.tile_pool(name="io", bufs=1) as iop, tc.tile_pool(
        name="op", bufs=OUT_BUFS
    ) as outp:
        lt = iop.tile([P, NT * W], f32)
        rt = iop.tile([P, NT * W], f32)
        l3 = lt[:].rearrange("p (t w) -> p t w", t=NT, w=W)
        r3 = rt[:].rearrange("p (t w) -> p t w", t=NT, w=W)
        lsrc = left[:].rearrange("bc h w -> (bc h) w").rearrange(
            "(t p) w -> p t w", p=P
        )
        rsrc = right[:].rearrange("bc h w -> (bc h) w").rearrange(
            "(t p) w -> p t w", p=P
        )
        # per-block input loads so the first compute starts after block 0 lands
        for t in range(NT):
            nc.sync.dma_start(out=l3[:, t, :], in_=lsrc[:, t, :])
            nc.sync.dma_start(out=r3[:, t, :], in_=rsrc[:, t, :])

        for _ in range(rep):
            d0 = 0
            for gi, G in enumerate(GROUPS):
                ot = outp.tile([P, GMAX * NT * W], f32, tag="out")
                o4 = ot[:].rearrange("p (g t w) -> p g t w", g=GMAX, t=NT, w=W)
                for g in range(G):
                    d = d0 + g
                    if d > d0:
                        # zero d0 <= w < d so the group rectangle DMA writes 0s
                        nc.gpsimd.memset(o4[:, g, :, d0:d], 0.0)
                    if gi == 0 and SPLIT_FIRST:
                        # leading group: per-block ops so compute starts on
                        # block 0 without waiting for all input DMAs
                        for t in range(NT):
                            nc.vector.tensor_sub(
                                o4[:, g, t, d:W],
                                l3[:, t, d:W],
                                r3[:, t, 0 : W - d],
                            )
                    else:
                        nc.vector.tensor_sub(
                            o4[:, g, :, d:W], l3[:, :, d:W], r3[:, :, 0 : W - d]
                        )
                for t in range(NT):
                    for p0, p1, bc, h0 in _pieces(t):
                        dest = out[
                            bc, d0 : d0 + G, h0 : h0 + (p1 - p0), d0:W
                        ].rearrange("d h w -> h d w")
                        nc.sync.dma_start(out=dest, in_=o4[p0:p1, 0:G, t, d0:W])
                d0 += G


def _build_nc(rep=1):
    import concourse.bacc as bacc
    import concourse.mybir as mybir
    from concourse import tile

    f32 = mybir.dt.float32
    nc = bacc.Bacc("TRN2")
    left = nc.dram_tensor("left", [BC_PER, H, W], f32, kind="ExternalInput")
    right = nc.dram_tensor("right", [BC_PER, H, W], f32, kind="ExternalInput")
    out = nc.dram_tensor("out", [BC_PER, MAX_DISP, H, W], f32, kind="ExternalOutput")

    with tile.TileContext(nc) as tc:
        build_body(nc, tc, left, right, out, rep=rep)
    nc.finalize()
    return nc


def _get_nc():
    if "nc" not in _NC_CACHE:
        _NC_CACHE["nc"] = _build_nc()
    return _NC_CACHE["nc"]


def run(left_feature, right_feature, **spmd_kwargs):
    """Run the SPMD kernel; returns (volume, BassKernelResults)."""
    from concourse.bass_utils import run_bass_kernel_spmd

    nc = _get_nc()
    lf = np.ascontiguousarray(np.asarray(left_feature), dtype=np.float32).reshape(
        BC, H, W
    )
    rf = np.ascontiguousarray(np.asarray(right_feature), dtype=np.float32).reshape(
        BC, H, W
    )
    in_maps = [
        {
            "left": np.ascontiguousarray(lf[k * BC_PER : (k + 1) * BC_PER]),
            "right": np.ascontiguousarray(rf[k * BC_PER : (k + 1) * BC_PER]),
        }
        for k in range(NCORES)
    ]
    res = run_bass_kernel_spmd(nc, in_maps, core_ids=list(range(NCORES)), **spmd_kwargs)
    chunks = [res.results[k]["out"] for k in range(NCORES)]
    vol = np.concatenate(chunks, axis=0).reshape(B, C, MAX_DISP, H, W)
    return vol, res


def kernel(left_feature, right_feature):
    vol, _ = run(left_feature, right_feature)
    return vol
